# revision 34
# baseline (speedup 1.0000x reference)
"""Multi-head self-attention (B=2, S=2048, D=1024, H=16, Dh=64) on 8 TRN2 cores.

Sharding: DP2 x TP4. Core c handles batch c//4 and heads 4*(c%4)..4*(c%4)+3.
Per core: Wq/Wk/Wv column slice [1024,256], Wo row slice [256,1024]; partial
outputs summed with per-query-group ReduceScatters, shards gathered on host.

Device layout (all matmul inputs bf16, PSUM fp32):
  - X^T (augmented with a ones row for the V bias) in SBUF [1025,2048].
  - Q^T,K^T feature-major [256,2048]; 1/sqrt(dh) folded into Wq/bq host-side;
    q/k biases applied per-partition during the ACT-engine PSUM drain.
  - V sequence-major per-128-row block as [128, 4*65] with a ones column per
    head so one matmul yields attn numerator + softmax denominator (row 64).
  - softmax without max-subtraction (scores ~ N(0,1), exp is safe).
  - head-pair score matmuls at lhsT base partitions 0/64 run concurrently on
    the PE (64-row tile groups).
  - denominator reciprocal on DVE, broadcast across partitions via a K=1 bf16
    matmul, copied to SBUF (PSUM single-read rule) before the normalize mul.
"""

import sys

import numpy as np
import ml_dtypes

sys.path.insert(0, "/opt/trn_rl_repo")

import concourse.bass as bass
import concourse.tile as tile
from concourse import mybir
from concourse.bass_utils import run_bass_kernel_spmd

B, S, D = 2, 2048, 1024
H, DH = 16, 64
HPC = 4               # heads per core
C = HPC * DH          # 256 feature cols per core
N_CORES = 8
GROUPS = [[0, 1, 2, 3], [4, 5, 6, 7]]
FP = mybir.dt.float32
BF = mybir.dt.bfloat16
BF_NP = ml_dtypes.bfloat16

KB = S // 128         # 16 key blocks of 128
QB = S // 512         # 4 query groups of 512
DC = D // 128         # 8 contraction chunks of 128
LEAD = 2              # attn-V matmul lags exp by LEAD rounds

_CACHE = {}


def _build(compiled=True, reps=1, phase="all"):
    from concourse.bacc import Bacc
    nc = Bacc(num_devices=N_CORES)
    xT_d = nc.declare_dram_parameter("xT", [D + 1, S], BF, isOutput=False)
    wq_d = nc.declare_dram_parameter("wq", [D, C], BF, isOutput=False)
    wk_d = nc.declare_dram_parameter("wk", [D, C], BF, isOutput=False)
    wv_d = nc.declare_dram_parameter("wv", [D + 1, C], BF, isOutput=False)
    wo_d = nc.declare_dram_parameter("wo", [C, D], BF, isOutput=False)
    bq_d = nc.declare_dram_parameter("bq2", [128, 2], FP, isOutput=False)
    bk_d = nc.declare_dram_parameter("bk2", [128, 2], FP, isOutput=False)
    out_d = nc.declare_dram_parameter("out", [S // 4, D], FP, isOutput=True)

    with tile.TileContext(nc) as tc:
        _emit(tc, xT_d, wq_d, wk_d, wv_d, wo_d, bq_d, bk_d, out_d, reps=reps,
              phase=phase)
    if compiled:
        nc.compile()
    return nc


def _emit(tc, xT_d, wq_d, wk_d, wv_d, wo_d, bq_d, bk_d, out_d, reps=1,
          phase="all"):
    nc = tc.nc
    ident = mybir.ActivationFunctionType.Identity
    with (
        tc.tile_pool(name="persist", bufs=1) as pp,
        tc.tile_pool(name="work", bufs=3) as wp,
        tc.tile_pool(name="psum", bufs=4, space="PSUM") as ps,
        tc.tile_pool(name="dram", bufs=1, space="DRAM") as dp,
    ):
        # ---- constants ----
        zbias = pp.tile([128, 1], FP, name="zbias", tag="zbias")
        nc.gpsimd.memset(zbias[:], 0.0)
        ones64 = pp.tile([1, 64], BF, name="ones64", tag="ones64")
        nc.gpsimd.memset(ones64[:], 1.0)

        # ---- load inputs ----
        xt = []
        for k in range(DC):
            t = pp.tile([128, S], BF, name=f"xt{k}", tag=f"xt{k}")
            nc.gpsimd.dma_start(t[:], xT_d[k * 128:(k + 1) * 128, :])
            xt.append(t)
        xta = pp.tile([1, S], BF, name="xta", tag="xta")
        nc.gpsimd.dma_start(xta[:], xT_d[D:D + 1, :])

        ws = {}
        for wname, wd in (("wq", wq_d), ("wk", wk_d), ("wv", wv_d)):
            chunks = []
            for k in range(DC):
                t = pp.tile([128, C], BF, name=f"{wname}{k}", tag=f"{wname}{k}")
                nc.gpsimd.dma_start(t[:], wd[k * 128:(k + 1) * 128, :])
                chunks.append(t)
            ws[wname] = chunks
        vta = pp.tile([1, C], BF, name="wva", tag="wva")
        nc.gpsimd.dma_start(vta[:], wv_d[D:D + 1, :])

        wo = []
        for k in range(2):
            t = pp.tile([128, D], BF, name=f"wo{k}", tag=f"wo{k}")
            nc.gpsimd.dma_start(t[:], wo_d[k * 128:(k + 1) * 128, :])
            wo.append(t)

        bq_t = pp.tile([128, 2], FP, name="bq_t", tag="bq_t")
        nc.gpsimd.dma_start(bq_t[:], bq_d[:, :])
        bk_t = pp.tile([128, 2], FP, name="bk_t", tag="bk_t")
        nc.gpsimd.dma_start(bk_t[:], bk_d[:, :])

        # ---- persistent activations ----
        qt = [pp.tile([128, S], BF, name=f"qt{r}", tag=f"qt{r}") for r in range(2)]
        kt = [pp.tile([128, S], BF, name=f"kt{r}", tag=f"kt{r}") for r in range(2)]
        at = [pp.tile([128, S], BF, name=f"at{r}", tag=f"at{r}") for r in range(2)]
        va = []
        for k in range(KB):
            t = pp.tile([128, HPC * (DH + 1)], BF, name=f"va{k}", tag=f"va{k}")
            nc.gpsimd.memset(t[:], 1.0)
            va.append(t)

        rs_in = [dp.tile([512, D], FP, name=f"rsin{q}", tag=f"rsin{q}")
                 for q in range(QB)]
        rs_out = [dp.tile([128, D], FP, name=f"rsout{q}", tag=f"rsout{q}")
                  for q in range(QB)]

        # ---- QKV projections ----
        # Q^T, K^T: [256 feat, 2048 seq] as 2 row tiles; bias folded into the
        # ACT drain (per-partition bias in feature-major layout).
        def emit_qkv():
            for wname, dst, bias_t in (("wq", qt, bq_t), ("wk", kt, bk_t)):
                chunks = ws[wname]
                for rb in range(2):
                    for cbp in range(QB // 2):
                        psq = ps.tile([128, 1024], FP, name="psq", tag="mm",
                                      bufs=2)
                        for j in range(2):
                            cb = 2 * cbp + j
                            for k in range(DC):
                                nc.tensor.matmul(
                                    psq[:, j * 512:(j + 1) * 512],
                                    chunks[k][:, rb * 128:(rb + 1) * 128],
                                    xt[k][:, cb * 512:(cb + 1) * 512],
                                    start=(k == 0), stop=(k == DC - 1),
                                )
                        nc.scalar.activation(
                            dst[rb][:, cbp * 1024:(cbp + 1) * 1024], psq[:],
                            ident, bias=bias_t[:, rb:rb + 1],
                        )

            # V: sequence-major, bias via the augmented ones row of X^T.
            vchunks = ws["wv"]
            for sbg in range(KB // 4):
                psv = ps.tile([128, 1024], FP, name="psv", tag="mm", bufs=2)
                for j in range(4):
                    sb = 4 * sbg + j
                    vsl = slice(j * C, (j + 1) * C)
                    for k in range(DC):
                        nc.tensor.matmul(
                            psv[:, vsl],
                            xt[k][:, sb * 128:(sb + 1) * 128],
                            vchunks[k][:],
                            start=(k == 0), stop=False,
                        )
                    nc.tensor.matmul(
                        psv[:, vsl], xta[:, sb * 128:(sb + 1) * 128], vta[:],
                        start=False, stop=True,
                    )
                for j in range(4):
                    sb = 4 * sbg + j
                    for h in range(HPC):
                        nc.vector.tensor_copy(
                            va[sb][:, h * 65:h * 65 + 64],
                            psv[:, j * C + h * 64:j * C + (h + 1) * 64],
                        )

        # ---- attention + output projection + reduce-scatter ----
        def emit_pair(qb, ht, mode="full"):
            qsl = slice(qb * 512, (qb + 1) * 512)
            if mode in ("atonly", "at128"):
                m = 128 if mode == "at128" else 65
                psa = [ps.tile([m, 512], FP, name=f"psa{hr}", tag="psa",
                               bufs=2) for hr in range(2)]
                for kb in range(KB):
                    for hr in range(2):
                        h = 2 * ht + hr
                        sl = (slice(0, 128) if mode == "at128"
                              else slice(h * 65, h * 65 + 65))
                        nc.tensor.matmul(
                            psa[hr][:], va[kb][:, sl], kt[ht][:, qsl],
                            start=(kb == 0), stop=(kb == KB - 1),
                        )
                for hr in range(2):
                    dead = wp.tile([m, 512], FP, name="dead", tag="dead",
                                   bufs=2)
                    nc.vector.tensor_copy(dead[:], psa[hr][:])
                return
            psa = [ps.tile([65, 512], FP, name=f"psa{hr}", tag="psa", bufs=2)
                   for hr in range(2)]

            def emit_at(r, ptb):
                for hr in range(2):
                    h = 2 * ht + hr
                    nc.tensor.matmul(
                        psa[hr][:],
                        va[r][:, h * 65:h * 65 + 65],
                        ptb[:, hr * 512:(hr + 1) * 512],
                        start=(r == 0), stop=(r == KB - 1),
                    )

            pts = []
            for kb in range(KB):
                pss = ps.tile([128, 1024], FP, name="pss", tag="mm", bufs=2)
                for hr in range(2):
                    rows = slice(hr * 64, (hr + 1) * 64)
                    nc.tensor.matmul(
                        pss[:, hr * 512:(hr + 1) * 512],
                        kt[ht][rows, kb * 128:(kb + 1) * 128],
                        qt[ht][rows, qsl],
                    )
                if mode == "sconly":
                    continue
                if mode in ("full", "nonorm", "mixed") and kb >= LEAD:
                    emit_at(kb - LEAD, pts[kb - LEAD])
                ptb = wp.tile([128, 1024], BF, name="pt", tag="pt",
                              bufs=LEAD + 2)
                if mode == "mixed":
                    nc.vector.tensor_copy(ptb[:], pss[:])
                else:
                    nc.scalar.activation(
                        ptb[:], pss[:], mybir.ActivationFunctionType.Exp,
                        bias=zbias[:],
                    )
                pts.append(ptb)
            if mode == "sc" or mode == "sconly":
                return
            for r in range(max(0, KB - LEAD), KB):
                emit_at(r, pts[r])
            if mode in ("nonorm", "mixed"):
                for hr in range(2):
                    dead = wp.tile([65, 512], FP, name="dead", tag="dead",
                                   bufs=2)
                    nc.vector.tensor_copy(dead[:], psa[hr][:])
                return
            for hr in range(2):
                rows = slice(hr * 64, (hr + 1) * 64)
                recipf = wp.tile([1, 512], FP, name="recipf", tag="recipf",
                                 bufs=2)
                nc.vector.reciprocal(recipf[:], psa[hr][64:65, :])
                recipb = wp.tile([1, 512], BF, name="recipb", tag="recipb",
                                 bufs=2)
                nc.vector.tensor_copy(recipb[:], recipf[:])
                psb = ps.tile([64, 512], FP, name="psb", tag="tail", bufs=2)
                nc.tensor.matmul(psb[:], ones64[:], recipb[:])
                psbs = wp.tile([64, 512], FP, name="psbs", tag="psbs", bufs=2)
                nc.vector.tensor_copy(psbs[:], psb[:])
                nc.vector.tensor_mul(
                    at[ht][rows, qsl], psa[hr][0:64, :], psbs[:])

        def emit_oproj(qb, js):
            for j in js:
                q0 = qb * 512 + j * 128
                for ob in range(2):
                    pso = ps.tile([128, 512], FP, name="pso", tag="tail",
                                  bufs=2)
                    nc.tensor.matmul(
                        pso[:], at[0][:, q0:q0 + 128],
                        wo[0][:, ob * 512:(ob + 1) * 512],
                        start=True, stop=False,
                    )
                    nc.tensor.matmul(
                        pso[:], at[1][:, q0:q0 + 128],
                        wo[1][:, ob * 512:(ob + 1) * 512],
                        start=False, stop=True,
                    )
                    osb = wp.tile([128, 512], FP, name="osb", tag="osb")
                    nc.vector.tensor_copy(osb[:], pso[:])
                    nc.gpsimd.dma_start(
                        rs_in[qb][j * 128:(j + 1) * 128,
                                  ob * 512:(ob + 1) * 512],
                        osb[:])

        def emit_rs(qb):
            nc.gpsimd.collective_compute(
                "ReduceScatter",
                mybir.AluOpType.add,
                replica_groups=GROUPS,
                ins=[rs_in[qb].opt()],
                outs=[rs_out[qb].opt()],
            )
            nc.gpsimd.dma_start(out_d[qb * 128:(qb + 1) * 128, :],
                                rs_out[qb][:])

        def body_all():
            emit_qkv()
            for qb in range(QB):
                emit_pair(qb, 0)
                if qb > 0:
                    emit_oproj(qb - 1, [0, 1])
                emit_pair(qb, 1)
                if qb > 0:
                    emit_oproj(qb - 1, [2, 3])
                    if reps == 1:
                        emit_rs(qb - 1)
            emit_oproj(QB - 1, [0, 1])
            emit_oproj(QB - 1, [2, 3])
            if reps == 1:
                emit_rs(QB - 1)

        if phase in ("attn", "oproj", "sc", "sconly", "nonorm", "atonly", "at128", "mixed"):
            emit_qkv()

        if reps > 1:
            _loop_cm = tc.For_i(0, reps, 1)
            _loop_cm.__enter__()

        if phase == "all":
            body_all()
        elif phase == "qkv":
            emit_qkv()
        elif phase == "attn":
            for qb in range(QB):
                emit_pair(qb, 0)
                emit_pair(qb, 1)
        elif phase in ("sc", "sconly", "nonorm", "atonly", "at128", "mixed"):
            for qb in range(QB):
                emit_pair(qb, 0, mode=phase)
                emit_pair(qb, 1, mode=phase)
        elif phase == "oproj":
            for qb in range(QB):
                emit_oproj(qb, [0, 1])
                emit_oproj(qb, [2, 3])

        if reps > 1:
            _loop_cm.__exit__(None, None, None)
            for qb in range(QB):
                emit_rs(qb)


def _get_nc(compiled=True, reps=1, phase="all"):
    key = ("ncc" if compiled else "nc", reps, phase, LEAD)
    if key not in _CACHE:
        _CACHE[key] = _build(compiled, reps, phase)
    return _CACHE[key]


def _in_maps(inputs, Wq, bq, Wk, bk, Wv, bv, Wo, bo):
    scale = 1.0 / np.sqrt(DH)
    ones = np.ones((1, S), np.float32)
    xts = []
    for b in range(B):
        xts.append(np.concatenate(
            [np.ascontiguousarray(inputs[b].T), ones], axis=0).astype(BF_NP))
    maps = []
    for c in range(N_CORES):
        b, hg = divmod(c, 4)
        cols = slice(hg * C, (hg + 1) * C)
        wv_aug = np.concatenate([Wv[:, cols], bv[cols][None, :]], axis=0)
        maps.append({
            "xT": xts[b],
            "wq": (Wq[:, cols] * scale).astype(BF_NP),
            "wk": np.ascontiguousarray(Wk[:, cols]).astype(BF_NP),
            "wv": wv_aug.astype(BF_NP),
            "wo": np.ascontiguousarray(Wo[cols, :]).astype(BF_NP),
            "bq2": np.ascontiguousarray(
                (bq[cols] * scale).reshape(2, 128).T.astype(np.float32)),
            "bk2": np.ascontiguousarray(
                bk[cols].reshape(2, 128).T.astype(np.float32)),
        })
    return maps


def _gather(results, bo):
    out = np.empty((B, S, D), np.float32)
    for c in range(N_CORES):
        b, rank = divmod(c, 4)
        o = results[c]
        for qb in range(QB):
            out[b, qb * 512 + rank * 128:qb * 512 + (rank + 1) * 128, :] = \
                o[qb * 128:(qb + 1) * 128]
    out += bo.astype(np.float32)[None, None, :]
    return out


def _run(inputs, Wq, bq, Wk, bk, Wv, bv, Wo, bo, **run_kwargs):
    nc = _get_nc()
    maps = _in_maps(inputs, Wq, bq, Wk, bk, Wv, bv, Wo, bo)
    res = run_bass_kernel_spmd(nc, maps, core_ids=list(range(N_CORES)),
                               **run_kwargs)
    out = _gather([res.results[c]["out"] for c in range(N_CORES)], bo)
    return out, res


def kernel(inputs, Wq, bq, Wk, bk, Wv, bv, Wo, bo):
    out, _ = _run(inputs, Wq, bq, Wk, bk, Wv, bv, Wo, bo)
    return out
